# revision 1
# baseline (speedup 1.0000x reference)
"""DigitCaps dynamic-routing kernel for 8x TRN2 NeuronCores.

Data-parallel over batch (512 -> 64 per core). Per core, the routing loop is
restructured so u_hat (B*O*I*D) is never materialized:

  s0      = 0.1 * x @ W            (PE, K=i*d contraction)
  v0      = squash(s0)
  bval_t  = UV(v_t):  A = W x_D v (PE per o-pair), bval = sum_d(A * x) (DVE)
  c_t     = softmax_o(bval_t)      (PE bridge transpose -> ACT exp -> DVE)
  s_t     = sum_i c*u_hat via y=c*x (DVE) then PE matmul over K=i per d
  v_t     = squash(s_t)

b2 = b1 + UV(v1) = UV(v0+v1) by linearity, so each iteration is one UV pass.
All PE inputs bf16, fp32 PSUM accumulate.
"""

import numpy as np
import ml_dtypes

import concourse.bass as bass
import concourse.bacc as bacc
import concourse.mybir as mybir
from concourse.tile import TileContext
from concourse.bass_utils import run_bass_kernel_spmd

bf16 = ml_dtypes.bfloat16
F32 = mybir.dt.float32
BF = mybir.dt.bfloat16
AF = mybir.ActivationFunctionType
ALU = mybir.AluOpType
AX = mybir.AxisListType

B, O, I, D, d = 512, 10, 1152, 16, 8
BL = 64          # batch per core
NPAIR = 5        # o-pairs
NQ = 9           # i chunks of 128
NT = 72          # (q, d) tiles
PI_2 = float(np.pi / 2)
import os
_SKIP = set(os.environ.get("KABL", "").split(","))
_B = lambda k, dflt: int(os.environ.get(k, dflt))
_DEFAULTS = {"ASB": 3, "YB": 4, "BLKDVE": 1, "EVD": 0}


def _squash(nc, pool, z_ap, shape, tag, pihalf):
    """v = z - 1 - gelu(z) + cos(z) - relu(z); returns fp32 tile."""
    g = pool.tile(shape, F32, tag=f"{tag}_g")
    si = pool.tile(shape, F32, tag=f"{tag}_s")
    r = pool.tile(shape, F32, tag=f"{tag}_r")
    v = pool.tile(shape, F32, tag=f"{tag}_v")
    nc.scalar.activation(g[:], z_ap, AF.Gelu)
    nc.scalar.activation(si[:], z_ap, AF.Sin, bias=pihalf[0:shape[0]])  # cos(z)
    nc.scalar.activation(r[:], z_ap, AF.Relu)
    # v = (z - 1) - g
    nc.vector.scalar_tensor_tensor(v[:], z_ap, 1.0, g[:], ALU.subtract, ALU.subtract)
    nc.vector.tensor_tensor(v[:], v[:], si[:], ALU.add)
    nc.vector.tensor_tensor(v[:], v[:], r[:], ALU.subtract)
    return v


def _body(nc, tc, x3_d, xdi_d, w2_d, w1s_d, idb_d, id32_d, msk_d, out_d):
    with (
        tc.tile_pool(name="const", bufs=1) as cpool,
        tc.tile_pool(name="work", bufs=1) as wpool,
        tc.tile_pool(name="small", bufs=2) as spool,
        tc.tile_pool(name="psMM", bufs=_B("PSMM", 2), space="PSUM") as psMM_pool,
        tc.tile_pool(name="psA2", bufs=_B("PSA2", 2), space="PSUM") as psA2_pool,
        tc.tile_pool(name="psS", bufs=1, space="PSUM") as psS_pool,
    ):
        # ---- resident loads (split for DMA parallelism) ----
        x3 = cpool.tile([128, NQ, d, BL], BF)
        xdi = cpool.tile([128, d, I], BF)
        w1s = cpool.tile([128, NT, 160], BF)
        idb = cpool.tile([128, 128], BF)
        id32 = cpool.tile([64, 64], F32)
        for q0 in range(0, NQ, 3):
            nc.sync.dma_start(x3[:, q0:q0 + 3], x3_d.ap()[:, q0:q0 + 3])
        for d0 in range(0, d, 2):
            nc.sync.dma_start(xdi[:, d0:d0 + 2], xdi_d.ap()[:, d0:d0 + 2])
        for t0 in range(0, NT, 18):
            nc.sync.dma_start(w1s[:, t0:t0 + 18], w1s_d.ap()[:, t0:t0 + 18])
        nc.sync.dma_start(idb[:], idb_d.ap())
        nc.sync.dma_start(id32[:], id32_d.ap())
        msk = cpool.tile([128, 2], F32)
        nc.sync.dma_start(msk[:], msk_d.ap())
        pihalf = cpool.tile([128, 1], F32)
        nc.gpsimd.memset(pihalf[:], PI_2)

        # ---- phase s0: s0[b,(o,D)] = sum_{i,d} x*W ----
        ps0 = psMM_pool.tile([BL, 160], F32, tag="mm")
        for t in range(NT):
            q, d_ = divmod(t, d)
            nc.tensor.matmul(ps0[:], x3[:, q, d_], w1s[:, t],
                             start=(t == 0), stop=(t == NT - 1))
        z0 = wpool.tile([BL, 160], F32, tag="z0")
        nc.scalar.mul(z0[:], ps0[:], 0.1)
        v0b = _squash(nc, wpool, z0[:], [BL, 160], "sq0", pihalf)  # [64,160] b-world

        # transpose v0 per pair -> vT [32, (p,b)]
        vT = wpool.tile([32, NPAIR, BL], F32, tag="vT")
        for p in range(NPAIR):
            pst = psMM_pool.tile([32, BL], F32, tag="mm", name=f"pst0_{p}")
            nc.tensor.transpose(pst[:], v0b[:, 32 * p:32 * p + 32], id32[:])
            nc.scalar.copy(vT[:, p], pst[:])
        vsum = wpool.tile([32, NPAIR, BL], F32, tag="vsum")
        nc.vector.tensor_copy(vsum[:], vT[:])

        vfinal = None
        for it in range(2):
            vin = vT if it == 0 else vsum
            # block-diag lhsT per pair [32, 128]
            blk = wpool.tile([32, NPAIR, 128], BF, tag="blk")
            for p in range(NPAIR):
                (nc.vector.tensor_scalar(blk[:, p, 0:64], vin[:, p], msk[0:32, 0:1], None, ALU.mult)
                 if _B("BLKDVE", 1) else
                 nc.scalar.mul(blk[:, p, 0:64], vin[:, p], msk[0:32, 0:1]))
                (nc.vector.tensor_scalar(blk[:, p, 64:128], vin[:, p], msk[0:32, 1:2], None, ALU.mult)
                 if _B("BLKDVE", 1) else
                 nc.scalar.mul(blk[:, p, 64:128], vin[:, p], msk[0:32, 1:2]))

            # prefetch the exp table set while ACT is otherwise idle
            dummy = wpool.tile([1, 1], F32, tag="dummy", bufs=2)
            nc.scalar.activation(dummy[:], pihalf[0:1], AF.Exp)
            # ---- UV: A + evict + mult + d-tree -> bval [128,(p,i)] ----
            bval = wpool.tile([128, NPAIR, I], BF, tag="bval")
            for p in range(NPAIR):
                w2t = spool.tile([32, d * I], BF, tag="w2t", bufs=_B("W2T", 2))
                ws = _B("WSPLIT", 2)
                for d0 in range(0, d, ws):
                    nc.sync.dma_start(
                        w2t[:, d0 * I:(d0 + ws) * I],
                        w2_d.ap()[:, p, d0:d0 + ws].rearrange("p a b -> p (a b)"))
                A_sb = wpool.tile([128, d * I], BF, tag="A_sb", bufs=_B("ASB", 3))
                for n in range(9 if "amm" not in _SKIP else 0):
                    psA = psA2_pool.tile([128, 1024], F32, tag="psA", name=f"psA{it}_{p}_{n}")
                    nc.tensor.matmul(psA[:, 0:512], blk[:, p],
                                     w2t[:, 1024 * n:1024 * n + 512], start=True, stop=True)
                    nc.tensor.matmul(psA[:, 512:1024], blk[:, p],
                                     w2t[:, 1024 * n + 512:1024 * (n + 1)], start=True, stop=True)
                    if p < _B("EVD", 0):
                        nc.vector.tensor_copy(A_sb[:, 1024 * n:1024 * (n + 1)], psA[:])
                    else:
                        nc.scalar.copy(A_sb[:, 1024 * n:1024 * (n + 1)], psA[:])
                # in-place: prod and the d-reduction tree reuse A_sb columns
                if "tree" in _SKIP: continue
                nc.vector.tensor_tensor(A_sb[:], A_sb[:], xdi[:].rearrange("p a b -> p (a b)"), ALU.mult)
                nc.vector.tensor_tensor(A_sb[:, 0:4 * I], A_sb[:, 0:4 * I], A_sb[:, 4 * I:8 * I], ALU.add)
                nc.vector.tensor_tensor(A_sb[:, 0:2 * I], A_sb[:, 0:2 * I], A_sb[:, 2 * I:4 * I], ALU.add)
                nc.vector.tensor_tensor(bval[:, p], A_sb[:, 0:I], A_sb[:, I:2 * I], ALU.add)

            # ---- bridge: transpose bval -> bvalT [128=i, (q),(o,b)] ----
            # transpose psums are consumed directly by exp below (no staging)
            pstq = []
            for q in range(NQ if "bridge" not in _SKIP else 0):
                pst5 = psMM_pool.tile([128, 640], BF, tag="mm", name=f"pstb{it}_{q}")
                for p in range(NPAIR):
                    nc.tensor.transpose(pst5[:, 128 * p:128 * (p + 1)],
                                        bval[:, p, 128 * q:128 * (q + 1)], idb[:])
                pstq.append(pst5)

            # ---- softmax over o: cT = exp(bval); 1/Z folds into xs = x * r ----
            cT = wpool.tile([128, NQ, O, BL], BF, tag="cT")
            xs = wpool.tile([128, NQ, d, BL], BF, tag="xs")
            for q in range(NQ if "smax" not in _SKIP else 0):
                nc.scalar.activation(cT[:, q],
                                     pstq[q][:].rearrange("p (o b) -> p o b", o=O), AF.Exp)
                Z = spool.tile([128, BL], F32, tag="Z")
                nc.vector.tensor_reduce(Z[:], cT[:, q].rearrange("p o b -> p b o"),
                                        AX.X, ALU.add)
                rec = spool.tile([128, BL], F32, tag="rec")
                nc.vector.reciprocal(rec[:], Z[:])
                nc.vector.tensor_tensor(xs[:, q], x3[:, q],
                                        rec[:].unsqueeze(1).broadcast_to((128, d, BL)),
                                        ALU.mult)

            # prefetch gelu/sin table sets for the upcoming squash
            dummy2 = wpool.tile([1, 1], F32, tag="dummy2", bufs=2)
            nc.scalar.activation(dummy2[:], pihalf[0:1], AF.Gelu)
            dummy3 = wpool.tile([1, 1], F32, tag="dummy3", bufs=2)
            nc.scalar.activation(dummy3[:], pihalf[0:1], AF.Sin)
            # ---- y = c * x, S matmuls (accumulate over 72 (q,d) tiles) ----
            psS8 = psS_pool.tile([128, 512], F32, tag="psS8", name=f"psS8{it}")
            psS9 = psS_pool.tile([32, 128], F32, tag="psS9", name=f"psS9{it}")
            for th in range(NT // 2 if "ys" not in _SKIP else 0):
                q, dh = divmod(th, d // 2)
                y = spool.tile([128, 2, O, BL], BF, tag="y", bufs=_B("YB", 6))
                nc.vector.tensor_tensor(
                    y[:], cT[:, q].unsqueeze(1).broadcast_to((128, 2, O, BL)),
                    xs[:, q, 2 * dh:2 * dh + 2].unsqueeze(2).broadcast_to((128, 2, O, BL)),
                    ALU.mult)
                for h in range(2 if "smm" not in _SKIP else 0):
                    t = q * d + 2 * dh + h
                    yf = y[:, h].rearrange("p o b -> p (o b)")
                    nc.tensor.matmul(psS8[:], w1s[:, t, 0:128], yf[:, 0:512],
                                     start=(t == 0), stop=(t == NT - 1))
                    nc.tensor.matmul(psS9[:], w1s[:, t, 128:160], yf[:, 512:640],
                                     start=(t == 0), stop=(t == NT - 1))

            # ---- extract diag s, squash ----
            sT = wpool.tile([32, NPAIR, BL], F32, tag="sT")
            sTa = wpool.tile([32, NPAIR, BL], F32, tag="sTa")
            for p in range(4):
                nc.scalar.mul(sTa[:, p], psS8[32 * p:32 * p + 32, 128 * p:128 * p + 64],
                              msk[32 * p:32 * p + 32, 0:1])
                nc.scalar.mul(sT[:, p], psS8[32 * p:32 * p + 32, 128 * p + 64:128 * p + 128],
                              msk[32 * p:32 * p + 32, 1:2])
            nc.scalar.mul(sTa[:, 4], psS9[:, 0:64], msk[0:32, 0:1])
            nc.scalar.mul(sT[:, 4], psS9[:, 64:128], msk[0:32, 1:2])
            nc.vector.tensor_tensor(sT[:], sT[:], sTa[:], ALU.add)
            vnew = _squash(nc, wpool, sT[:], [32, NPAIR, BL], "sqi", pihalf)
            if it == 0:
                nc.vector.tensor_tensor(vsum[:], vsum[:], vnew[:], ALU.add)
            else:
                vfinal = vnew

        # ---- output: vfinal [32=(o2,D), (p,b)] -> out[b, 2p+o2, D] ----
        out_ap = out_d.ap().rearrange("b (p o2) DD -> (o2 DD) p b", p=NPAIR, o2=2)
        for p in range(NPAIR):
            nc.sync.dma_start(out_ap[:, p], vfinal[:, p])


def build_program():
    nc = bacc.Bacc("TRN2", debug=False, target_bir_lowering=False)
    x3_d = nc.dram_tensor("x3", [128, NQ, d, BL], BF, kind="ExternalInput")
    xdi_d = nc.dram_tensor("xdi", [128, d, I], BF, kind="ExternalInput")
    w2_d = nc.dram_tensor("w2", [32, NPAIR, d, I], BF, kind="ExternalInput")
    w1s_d = nc.dram_tensor("w1s", [128, NT, 160], BF, kind="ExternalInput")
    idb_d = nc.dram_tensor("idb", [128, 128], BF, kind="ExternalInput")
    msk_d = nc.dram_tensor("msk", [128, 2], F32, kind="ExternalInput")
    id32_d = nc.dram_tensor("id32", [64, 64], F32, kind="ExternalInput")
    out_d = nc.dram_tensor("out", [BL, O, D], F32, kind="ExternalOutput")
    with TileContext(nc) as tc:
        _body(nc, tc, x3_d, xdi_d, w2_d, w1s_d, idb_d, id32_d, msk_d, out_d)
    nc.compile()
    return nc


def host_prep_w(W):
    """W: [1,10,1152,16,8] fp32 -> (w2, w1s, idb, id32) arrays."""
    Wb = W[0].astype(bf16)
    w2 = np.ascontiguousarray(
        Wb.reshape(5, 2, I, D, d).transpose(1, 3, 0, 4, 2)).reshape(32, NPAIR, d, I)
    w1s = np.ascontiguousarray(
        Wb.reshape(5, 2, NQ, 128, D, d).transpose(3, 2, 5, 0, 1, 4)).reshape(128, NT, 160)
    idb = np.eye(128, dtype=bf16)
    id32 = np.eye(64, dtype=np.float32)
    msk = np.zeros((128, 2), np.float32)
    msk[:, 0] = np.tile(np.r_[np.ones(16), np.zeros(16)], 4)
    msk[:, 1] = 1.0 - msk[:, 0]
    return w2, w1s, idb, id32, msk


def host_prep_x(xc):
    """xc: [64, 1152, 8] fp32 -> (x3, xdi)."""
    xb = xc.astype(bf16)
    x3 = np.ascontiguousarray(xb.reshape(BL, NQ, 128, d).transpose(2, 1, 3, 0))
    xd = np.ascontiguousarray(xb.transpose(0, 2, 1))        # [64, 8, 1152]
    xdi = np.concatenate([xd, xd], axis=0)                  # [128, 8, 1152]
    return x3, xdi


_NC_CACHE = {}


def _get_nc():
    if "nc" not in _NC_CACHE:
        _NC_CACHE["nc"] = build_program()
    return _NC_CACHE["nc"]


def kernel(x, W):
    x = np.asarray(x, dtype=np.float32)
    W = np.asarray(W, dtype=np.float32)
    w2, w1s, idb, id32, msk = host_prep_w(W)
    in_maps = []
    for core in range(8):
        x3, xdi = host_prep_x(x[core * BL:(core + 1) * BL])
        in_maps.append({"x3": x3, "xdi": xdi, "w2": w2, "w1s": w1s,
                        "idb": idb, "id32": id32, "msk": msk})
    nc = _get_nc()
    res = run_bass_kernel_spmd(nc, in_maps, list(range(8)))
    out = np.concatenate([res.results[i]["out"] for i in range(8)], axis=0)
    return out.astype(np.float32)

